# revision 3
# baseline (speedup 1.0000x reference)
"""Self-contained Bass/Trainium2 kernel for nn_Attention (B=4, N=2048, D=1024, H=16, dh=64).

Sharding: 8 cores = (batch b in 0..3) x (sequence half in 0..1).
Each core computes the full attention output for its 1024 rows of its batch:
full-sequence K/V are computed on-core (duplicated across the pair), so no
cross-core communication is needed. Host feeds x[b]^T with the core's own rows
last so one SPMD program serves all cores; softmax is order-invariant in j.

Layout: all matmul operands fp16 (PSUM f32). V is projected directly in
keys-major layout (stationary = x^T blocks, moving = Wv) so no PE transposes
are needed. Each V block carries 64 ones columns, so the AV matmul emits the
softmax row-sums replicated across PSUM partitions 64..127 for free; the
normalization is then one reciprocal_approx_fast + one multiply on DVE.
Projection work is split into PSUM-tile-sized units and interleaved into the
attention loop at key-tile granularity, keeping the PE continuously busy (and
ramped) while the ACT engine streams the exps. The first 4 of 8 contraction
blocks of the output projection (+bias) run inside the last attention group's
idle slots; only the last 4 trail the attention.

(A pair-wise DRAM AllGather variant that halves the K/V projection work was
measured at 623us vs 490us here: the 4MB collective costs ~122us on this
system, far exceeding the ~60us of duplicated projection it removes.)
"""

import sys
import numpy as np

sys.path.insert(0, "/opt/trn_rl_repo")

B, N, DIM = 4, 2048, 1024
HEADS, DH = 16, 64
SCALE = DH ** -0.5  # 0.125
NC = 8
HALF = N // 2  # rows per core

_compiled = None


def _build():
    import concourse.tile as tile
    from concourse import bacc, mybir

    f32 = mybir.dt.float32
    f16 = mybir.dt.float16
    EXP = mybir.ActivationFunctionType.Exp

    nc = bacc.Bacc("TRN2", target_bir_lowering=False, debug=False, num_devices=NC)

    X = nc.dram_tensor("x", (DIM, N), f16, kind="ExternalInput").ap()
    WQKV = nc.dram_tensor("w_qkv", (DIM, 3 * DIM), f16, kind="ExternalInput").ap()
    WOUT = nc.dram_tensor("w_out", (DIM, DIM), f16, kind="ExternalInput").ap()
    BOUT = nc.dram_tensor("b_out", (DIM,), f32, kind="ExternalInput").ap()
    Y = nc.dram_tensor("y", (HALF, DIM), f32, kind="ExternalOutput").ap()

    CT = DIM // 128   # 8 contraction tiles over channels
    MT = DIM // 128   # 8 dim tiles (head-pairs) for each of q,k
    JT = N // 128     # 16 key tiles
    VW = 128          # per-head v block: 64 dims + 64 ones columns

    with tile.TileContext(nc) as tc:
        with tc.tile_pool(name="persist", bufs=1) as persist, \
             tc.tile_pool(name="attnbuf", bufs=1) as attnbuf, \
             tc.tile_pool(name="wpool", bufs=3) as wpool:

            kT = [persist.tile([128, N], f16, tag="kT", bufs=MT, name=f"kT{m}")
                  for m in range(MT)]
            qT = [persist.tile([128, HALF], f16, tag="qT", bufs=MT,
                               name=f"qT{m}") for m in range(MT)]
            v_ext = [persist.tile([128, HEADS * VW], f16, tag="vext", bufs=JT,
                                  name=f"vext{t}") for t in range(JT)]
            ctx = [persist.tile([128, HALF], f16, tag="ctx", bufs=MT,
                                name=f"ctx{m}") for m in range(MT)]

            # bias broadcast to all partitions once
            bias_src = persist.tile([1, DIM], f32, tag="bias_src")
            nc.sync.dma_start(bias_src[:], BOUT.rearrange("(o d) -> o d", o=1))
            bias = persist.tile([128, DIM], f32, tag="bias")
            nc.gpsimd.partition_broadcast(bias[:], bias_src[0:1, :])

            # prefire the exp table load off the critical path
            dummy = attnbuf.tile([1, 8], f16, tag="dummy")
            nc.scalar.activation(dummy[:], bias_src[0:1, 0:8], EXP,
                                 bias=0.0, scale=1.0)

            # ones columns of v_ext (disjoint from the V-projection writes)
            for t in range(JT):
                ones_col = v_ext[t].rearrange(
                    "p (hh c) -> p hh c", c=VW)[:, :, DH:VW]
                nc.gpsimd.memset(ones_col, 1.0)

            with tc.tile_pool(name="psB", bufs=1, space="PSUM") as psB, \
                 tc.tile_pool(name="psInt", bufs=1, space="PSUM") as psInt:
                stage_cm = tc.tile_pool(name="stage", bufs=1)
                stage = stage_cm.__enter__()
                def w_col(base, m):
                    """[128, 8, 128] view of w_qkv[:, base+m*128 : +128]."""
                    return WQKV[:, base + m * 128:base + (m + 1) * 128].rearrange(
                        "(t p) d -> p t d", p=128)

                # ---- projection units: one PSUM-tile lifecycle each ----
                wt_cache = {}

                def get_wt(base, m):
                    key = (base, m)
                    if key not in wt_cache:
                        wt = wpool.tile([128, CT, 128], f16, tag="wkq",
                                        name=f"w{base}_{m}")
                        if not wt_cache:
                            # first tile: land the low ct blocks first so the
                            # opening matmul chain starts sooner
                            wc = w_col(base, m)
                            nc.sync.dma_start(wt[:, 0:2, :], wc[:, 0:2, :])
                            nc.sync.dma_start(wt[:, 2:CT, :], wc[:, 2:CT, :])
                        else:
                            nc.sync.dma_start(wt[:], w_col(base, m))
                        wt_cache[key] = wt
                    return wt_cache[key]

                # group-0 weight tiles first so the first K unit isn't stuck
                # behind the bulk x/wv transfers in the DMA queues
                for m in (0, 1):
                    get_wt(DIM, m)
                # x^T tiles (both halves) and Wv, resident through the last
                # projection unit
                xbT = [[stage.tile([128, HALF], f16, tag="xbT", bufs=2 * CT,
                                   name=f"xbT{h}_{ct}") for h in (0, 1)]
                       for ct in range(CT)]
                for ct in range(CT):
                    nc.sync.dma_start(
                        xbT[ct][0][:],
                        X[ct * 128:(ct + 1) * 128, 0:HALF])
                for m in (0, 1):
                    get_wt(0, m)
                for ct in range(CT):
                    nc.sync.dma_start(
                        xbT[ct][1][:],
                        X[ct * 128:(ct + 1) * 128, HALF:N])
                wv = [stage.tile([128, DIM], f16, tag="wv", bufs=CT,
                                 name=f"wv{ct}") for ct in range(CT)]
                for ct in range(CT):
                    nc.sync.dma_start(
                        wv[ct][:], WQKV[ct * 128:(ct + 1) * 128, 2 * DIM:3 * DIM])

                def kq_unit(base, m, h, s, dst, off):
                    """dst[:, off + s*512 : +512] = W[:, m-block].T @ x^T[h]."""
                    wt = get_wt(base, m)
                    ps = psInt.tile([128, 512], f32, tag="pint", bufs=2,
                                    name=f"pi{base}_{m}_{h}_{s}")
                    for ct in range(CT):
                        nc.tensor.matmul(ps[:],
                                         wt[:, ct, :],
                                         xbT[ct][h][:, s * 512:(s + 1) * 512],
                                         start=(ct == 0), stop=(ct == CT - 1))
                    nc.vector.tensor_copy(
                        dst[:, off + s * 512:off + (s + 1) * 512], ps[:])

                def v_unit(jtg, dc):
                    """v_ext[jtg] heads 8dc..8dc+7 from x^T block (keys-major)."""
                    h, kt = divmod(jtg, CT)
                    ps = psInt.tile([128, 512], f32, tag="pint", bufs=2,
                                    name=f"pv{jtg}_{dc}")
                    for ct in range(CT):
                        nc.tensor.matmul(
                            ps[:],
                            xbT[ct][h][:, kt * 128:(kt + 1) * 128],
                            wv[ct][:, dc * 512:(dc + 1) * 512],
                            start=(ct == 0), stop=(ct == CT - 1))
                    dst = v_ext[jtg].rearrange("p (hh c) -> p hh c", c=VW)[
                        :, 8 * dc:8 * dc + 8, 0:DH]
                    nc.vector.tensor_copy(dst, ps.rearrange(
                        "p (hh c) -> p hh c", c=DH))

                def group_units(g):
                    """Projection units for heads 4g..4g+3 (kT pairs 2g, 2g+1).
                    V is projected in 512-wide chunks (8 heads), carried by
                    groups 0 and 2."""
                    units = []
                    for m in (2 * g, 2 * g + 1):
                        for h in (0, 1):
                            for s in (0, 1):
                                units.append(lambda m=m, h=h, s=s: kq_unit(
                                    DIM, m, h, s, kT[m], h * HALF))
                    if g in (0, 2):
                        for jtg in range(JT):
                            units.append(lambda jtg=jtg, dc=g // 2: v_unit(
                                jtg, dc))
                    for m in (2 * g, 2 * g + 1):
                        for s in (0, 1):
                            units.append(lambda m=m, s=s: kq_unit(
                                0, m, 1, s, qT[m], 0))
                    return units

                pending = []

                # ---- attention for one head, interleaving pending units ----
                def attn_head(hd, interleave, pace=4):
                    hp, p = divmod(hd, 2)
                    po = psB.tile([128, HALF], f32, tag="po", bufs=1,
                                  name=f"po{hd}")
                    ats = {}

                    def av_half(j, s):
                        # one 512-half of the AV accumulation; interleaved with
                        # the score matmuls so consecutive PE stationaries
                        # alternate (kT, v_ext, kT, v_ext): a same-stationary
                        # pair's redundant 2nd LDWEIGHTS pins the background
                        # weight buffer and exposes ~95ns on the next matmul.
                        nc.tensor.matmul(
                            po[:, s * 512:(s + 1) * 512],
                            v_ext[j][:, hd * VW:(hd + 1) * VW],
                            ats[j][:, s * 512:(s + 1) * 512],
                            start=(j == 0), stop=(j == JT - 1))

                    for jt in range(JT):
                        pp = psB.tile([128, HALF], f32, tag="pp", bufs=2,
                                      name=f"pp{hd}_{jt}")
                        for s in (0, 1):
                            nc.tensor.matmul(
                                pp[:, s * 512:(s + 1) * 512],
                                kT[hp][p * 64:(p + 1) * 64,
                                       jt * 128:(jt + 1) * 128],
                                qT[hp][p * 64:(p + 1) * 64,
                                       s * 512:(s + 1) * 512],
                                start=True, stop=True)
                            if jt >= 1:
                                av_half(jt - 1, s)
                        at = attnbuf.tile([128, HALF], f16, tag="at", bufs=3,
                                          name=f"at{hd}_{jt}")
                        nc.scalar.activation(at[:], pp[:], EXP,
                                             bias=0.0, scale=SCALE)
                        ats[jt] = at
                        if jt >= 1:
                            ats.pop(jt - 1)
                        if interleave:
                            want = ((hd % 4) * JT + jt + 1) * interleave[0] \
                                // (pace * JT)
                            while interleave[0] - len(pending) < want and pending:
                                pending.pop(0)()
                    av_half(JT - 1, 0)
                    av_half(JT - 1, 1)
                    ats.pop(JT - 1)
                    # evacuate po with two quick copies so the next head's AV
                    # chain gets the PSUM bank back ASAP; the reciprocal and
                    # the normalizing multiply run off the critical path.
                    # (reciprocal_approx_fast mis-reads partition-offset PSUM
                    # APs, and SBUF*SBUF tensor ops need equal input base
                    # partitions, so both operands stage at partitions 0..63.)
                    ss = attnbuf.tile([64, HALF], f32, tag="ss", bufs=1,
                                      name=f"ss{hd}")
                    if hd == HEADS - 1:
                        nc.vector.tensor_copy(ss[:], po[64:128, :])
                        rb = attnbuf.tile([64, HALF], f32, tag="rb", bufs=1,
                                          name=f"rb{hd}")
                        nc.vector.reciprocal_approx_fast(rb[:], ss[:])
                        nc.vector.tensor_mul(ctx[hp][p * 64:(p + 1) * 64, :],
                                             po[0:64, :], rb[:])
                        return
                    cu = attnbuf.tile([64, HALF], f16, tag="cu", bufs=1,
                                      name=f"cu{hd}")
                    nc.vector.tensor_copy(cu[:], po[0:64, :])
                    nc.vector.tensor_copy(ss[:], po[64:128, :])
                    rb = attnbuf.tile([64, HALF], f32, tag="rb", bufs=1,
                                      name=f"rb{hd}")
                    nc.vector.reciprocal_approx_fast(rb[:], ss[:])
                    nc.vector.tensor_mul(ctx[hp][p * 64:(p + 1) * 64, :],
                                         cu[:], rb[:])

                # P0 up front; P(g+1) threads through A(g)'s slots
                for u in group_units(0):
                    u()
                for g in range(3):
                    if g < 2:
                        pending.extend(group_units(g + 1))
                    else:
                        pending.extend(group_units(3))
                    ileave = [len(pending)] if pending else None
                    for hd in range(4 * g, 4 * g + 4):
                        attn_head(hd, ileave)
                    while pending:
                        pending.pop(0)()

                stage_cm.__exit__(None, None, None)
                # stage (x^T, wv) is gone; w_out + the first half of the output
                # projection overlap A3, the tail reuses the psInt tiles
                with tc.tile_pool(name="outw", bufs=1) as outw:
                    wo = [outw.tile([128, DIM], f16, tag="wo", bufs=CT,
                                    name=f"wo{ft}") for ft in range(CT)]
                    yps = [outw.tile([128, DIM], f16, tag="yps", bufs=CT,
                                     name=f"yps{qt}") for qt in range(CT)]
                    for ft in range(CT):
                        nc.sync.dma_start(
                            wo[ft][:], WOUT[ft * 128:(ft + 1) * 128, :])

                    def yp_unit(qt, s, f0, src):
                        """dst s-half = src-half + sum(ft in f0..f0+3) ctx.T @ wo."""
                        ps = psInt.tile([128, 512], f32, tag="pint", bufs=2,
                                        name=f"py{qt}_{s}_{f0}")
                        for ft in range(f0, f0 + 4):
                            nc.tensor.matmul(ps[:],
                                             ctx[ft][:, qt * 128:(qt + 1) * 128],
                                             wo[ft][:, s * 512:(s + 1) * 512],
                                             start=(ft == f0), stop=(ft == f0 + 3))
                        sl = slice(s * 512, (s + 1) * 512)
                        if f0 == 0:
                            nc.vector.tensor_add(yps[qt][:, sl], ps[:], bias[:, sl])
                        else:
                            ysb = outw.tile([128, 512], f32, tag="ysb", bufs=3,
                                            name=f"ysb{qt}_{s}")
                            nc.vector.tensor_add(ysb[:], ps[:], yps[qt][:, sl])
                            nc.sync.dma_start(
                                Y[qt * 128:(qt + 1) * 128, sl], ysb[:])

                    pending.extend(lambda qt=qt, s=s: yp_unit(qt, s, 0, None)
                                   for qt in range(CT) for s in (0, 1))
                    ileave = [len(pending)]
                    for hd in range(12, 16):
                        attn_head(hd, ileave, pace=3)
                    while pending:
                        pending.pop(0)()
                    # tail: second half of the output projection
                    for qt in range(CT):
                        for s in (0, 1):
                            yp_unit(qt, s, 4, yps[qt])

    nc.compile()
    return nc


def _get_compiled():
    global _compiled
    if _compiled is None:
        _compiled = _build()
    return _compiled


def _build_in_maps(x, w_qkv, w_out, b_out):
    x = np.asarray(x, dtype=np.float32)
    w_qkv = np.ascontiguousarray(np.asarray(w_qkv, dtype=np.float16))
    w_out = np.ascontiguousarray(np.asarray(w_out, dtype=np.float16))
    b_out = np.asarray(b_out, dtype=np.float32)

    in_maps = []
    for c in range(NC):
        b, half = divmod(c, 2)
        other = x[b][(1 - half) * HALF:(2 - half) * HALF]
        mine = x[b][half * HALF:(half + 1) * HALF]
        xb = np.ascontiguousarray(
            np.concatenate([other, mine], axis=0).T.astype(np.float16))
        in_maps.append({"x": xb, "w_qkv": w_qkv, "w_out": w_out, "b_out": b_out})
    return in_maps


def kernel(x, w_qkv, w_out, b_out):
    from concourse.bass_utils import run_bass_kernel_spmd

    nc = _get_compiled()
    in_maps = _build_in_maps(x, w_qkv, w_out, b_out)
    res = run_bass_kernel_spmd(nc, in_maps, core_ids=list(range(NC)))

    out = np.empty((B, N, DIM), dtype=np.float32)
    for c in range(NC):
        b, half = divmod(c, 2)
        out[b, half * HALF:(half + 1) * HALF] = res.results[c]["y"]
    return out



# revision 6
# speedup vs baseline: 1.2960x; 1.2960x over previous
"""Self-contained Bass/Trainium2 kernel for nn_Attention (B=4, N=2048, D=1024, H=16, dh=64).

Sharding: 8 cores = (batch b in 0..3) x (head-group hg in 0..1), i.e. tensor
parallelism over heads inside each batch pair (per the to_qkv column / to_out
row sharding hint). Each core projects q/k/v for its 8 heads over the FULL
sequence (so no K/V duplication and no cross-core communication), runs
attention for those heads over all 2048 queries, and computes the row-sharded
half of the output projection. The two partial [2048, 1024] outputs of a pair
are summed on the host during unshard (bias is added on the hg=0 core; the
hg=1 core receives a zero bias so one SPMD program serves all cores).

This head split halves the K and V projection matmul work per core vs the
seq-split variant (which must build full-sequence K/V on both pair cores):
PE streaming work drops from ~918K to ~786K cycles/core.

Layout: all matmul operands fp16 (PSUM f32). V is projected directly in
keys-major layout (stationary = x^T blocks, moving = Wv) so no PE transposes
are needed. Each per-head V block carries 64 ones columns, so the AV matmul
emits the softmax row-sums replicated across PSUM partitions 64..127 for
free; the normalization is then one reciprocal_approx_fast + one multiply on
DVE. Projection work is split into PSUM-tile-sized units and interleaved into
the attention loop (V production streams inside the first attention unit at
one key-tile per step, just ahead of its consumption by the AV chain). The
output projection is split 3+1 over its 4 contraction tiles: the 3-tile half
(heads 0-5, ready after group 2) threads through the last attention group;
only the 1-tile tail trails the attention, overlapped with the output DMA.
"""

import sys
import numpy as np

sys.path.insert(0, "/opt/trn_rl_repo")

B, N, DIM = 4, 2048, 1024
HEADS, DH = 16, 64
NH = 8                # heads per core
SCALE = DH ** -0.5    # 0.125
NC = 8
HALF = N // 2

_compiled = None


def _build():
    import concourse.tile as tile
    from concourse import bacc, mybir

    f32 = mybir.dt.float32
    f16 = mybir.dt.float16
    EXP = mybir.ActivationFunctionType.Exp

    nc = bacc.Bacc("TRN2", target_bir_lowering=False, debug=False, num_devices=NC)

    X = nc.dram_tensor("x", (DIM, N), f16, kind="ExternalInput").ap()
    WQ = nc.dram_tensor("w_q", (DIM, NH * DH), f16, kind="ExternalInput").ap()
    WK = nc.dram_tensor("w_k", (DIM, NH * DH), f16, kind="ExternalInput").ap()
    WV = nc.dram_tensor("w_v", (DIM, NH * DH), f16, kind="ExternalInput").ap()
    WO = nc.dram_tensor("w_out", (NH * DH, DIM), f16, kind="ExternalInput").ap()
    BOUT = nc.dram_tensor("b_out", (DIM,), f32, kind="ExternalInput").ap()
    Y = nc.dram_tensor("y", (N, DIM), f16, kind="ExternalOutput").ap()

    CT = DIM // 128       # 8 contraction tiles over input channels
    MT = NH * DH // 128   # 4 dim tiles (head-pairs) for kT/qT/ctx
    JT = N // 128         # 16 key tiles
    VW = 128              # per-head v block: 64 dims + 64 ones columns

    with tile.TileContext(nc) as tc:
        with tc.tile_pool(name="persist", bufs=1) as persist, \
             tc.tile_pool(name="attnbuf", bufs=1) as attnbuf, \
             tc.tile_pool(name="wpool", bufs=1) as wpool:

            kT = [persist.tile([128, N], f16, tag="kT", bufs=MT, name=f"kT{m}")
                  for m in range(MT)]
            qT = [persist.tile([128, N], f16, tag="qT", bufs=MT,
                               name=f"qT{m}") for m in range(MT)]
            v_ext = [persist.tile([128, NH * VW], f16, tag="vext", bufs=JT,
                                  name=f"vext{t}") for t in range(JT)]
            ctx = [persist.tile([128, N], f16, tag="ctx", bufs=MT,
                                name=f"ctx{m}") for m in range(MT)]

            # bias broadcast to all partitions once (zeros on hg=1 cores)
            bias_src = persist.tile([1, DIM], f32, tag="bias_src")
            nc.sync.dma_start(bias_src[:], BOUT.rearrange("(o d) -> o d", o=1))
            bias = persist.tile([128, DIM], f32, tag="bias")
            nc.gpsimd.partition_broadcast(bias[:], bias_src[0:1, :])

            # prefire the exp table load off the critical path
            dummy = attnbuf.tile([1, 8], f16, tag="dummy")
            nc.scalar.activation(dummy[:], bias_src[0:1, 0:8], EXP,
                                 bias=0.0, scale=1.0)

            # ones columns of v_ext (disjoint from the V-projection writes)
            for t in range(JT):
                ones_col = v_ext[t].rearrange(
                    "p (hh c) -> p hh c", c=VW)[:, :, DH:VW]
                nc.gpsimd.memset(ones_col, 1.0)

            with tc.tile_pool(name="psB", bufs=1, space="PSUM") as psB, \
                 tc.tile_pool(name="psInt", bufs=1, space="PSUM") as psInt:
                stage_cm = tc.tile_pool(name="stage", bufs=1)
                stage = stage_cm.__enter__()

                def w_col(src, m):
                    """[128, 8, 128] view of src[:, m*128 : (m+1)*128]."""
                    return src[:, m * 128:(m + 1) * 128].rearrange(
                        "(t p) d -> p t d", p=128)

                # ---- weight + x staging, ordered for fastest first matmul ----
                wt_k = [wpool.tile([128, CT, 128], f16, tag="wkq",
                                   bufs=2 * MT, name=f"wk{m}") for m in range(MT)]
                wt_q = [wpool.tile([128, CT, 128], f16, tag="wkq",
                                   bufs=2 * MT, name=f"wq{m}") for m in range(MT)]
                xbT = [stage.tile([128, N], f16, tag="xbT", bufs=CT,
                                  name=f"xbT{ct}") for ct in range(CT)]
                wv = [stage.tile([128, 512], f16, tag="wv", bufs=CT,
                                 name=f"wv{ct}") for ct in range(CT)]

                # first K weight tile: low ct blocks first so the opening
                # matmul chain starts sooner
                wc = w_col(WK, 0)
                nc.sync.dma_start(wt_k[0][:, 0:2, :], wc[:, 0:2, :])
                nc.sync.dma_start(wt_k[0][:, 2:CT, :], wc[:, 2:CT, :])

                def dma_x_slice(s):
                    for ct in range(CT):
                        nc.sync.dma_start(
                            xbT[ct][:, s * 512:(s + 1) * 512],
                            X[ct * 128:(ct + 1) * 128, s * 512:(s + 1) * 512])

                dma_x_slice(0)
                nc.sync.dma_start(wt_q[0][:], w_col(WQ, 0))
                dma_x_slice(1)
                for ct in range(CT):
                    nc.sync.dma_start(wv[ct][:],
                                      WV[ct * 128:(ct + 1) * 128, :])
                dma_x_slice(2)
                nc.sync.dma_start(wt_k[1][:], w_col(WK, 1))
                nc.sync.dma_start(wt_q[1][:], w_col(WQ, 1))
                dma_x_slice(3)
                for m in (2, 3):
                    nc.sync.dma_start(wt_k[m][:], w_col(WK, m))
                    nc.sync.dma_start(wt_q[m][:], w_col(WQ, m))

                # ---- projection units: one PSUM-tile lifecycle each ----
                def kq_unit(wt, dst, s):
                    """dst[:, s*512 : +512] = W[:, m-block].T @ x^T."""
                    ps = psInt.tile([128, 512], f32, tag="pint", bufs=2,
                                    name=f"pi_{dst.name}_{s}")
                    for ct in range(CT):
                        nc.tensor.matmul(ps[:],
                                         wt[:, ct, :],
                                         xbT[ct][:, s * 512:(s + 1) * 512],
                                         start=(ct == 0), stop=(ct == CT - 1))
                    nc.vector.tensor_copy(
                        dst[:, s * 512:(s + 1) * 512], ps[:])

                def v_unit(t):
                    """v_ext[t] (all 8 heads) from x^T key block t."""
                    ps = psInt.tile([128, 512], f32, tag="pint", bufs=2,
                                    name=f"pv{t}")
                    for ct in range(CT):
                        nc.tensor.matmul(
                            ps[:],
                            xbT[ct][:, t * 128:(t + 1) * 128],
                            wv[ct][:],
                            start=(ct == 0), stop=(ct == CT - 1))
                    dst = v_ext[t].rearrange("p (hh c) -> p hh c", c=VW)[
                        :, :, 0:DH]
                    nc.vector.tensor_copy(dst, ps.rearrange(
                        "p (hh c) -> p hh c", c=DH))

                pending = []

                # ---- attention for one (head, query-half) unit ----
                def attn_unit(h, sq, interleave, u_idx, pace=4,
                              inline_v=False, last=False):
                    hp, p = divmod(h, 2)
                    po = psB.tile([128, 1024], f32, tag="po", bufs=1,
                                  name=f"po{h}_{sq}")
                    ats = {}

                    def av(j):
                        at = ats.pop(j)
                        for u in (0, 1):
                            nc.tensor.matmul(
                                po[:, u * 512:(u + 1) * 512],
                                v_ext[j][:, h * VW:(h + 1) * VW],
                                at[:, u * 512:(u + 1) * 512],
                                start=(j == 0), stop=(j == JT - 1))

                    for jt in range(JT):
                        pp = psB.tile([128, 1024], f32, tag="pp", bufs=2,
                                      name=f"pp{h}_{sq}_{jt}")
                        for u in (0, 1):
                            nc.tensor.matmul(
                                pp[:, u * 512:(u + 1) * 512],
                                kT[hp][p * 64:(p + 1) * 64,
                                       jt * 128:(jt + 1) * 128],
                                qT[hp][p * 64:(p + 1) * 64,
                                       sq * 1024 + u * 512:
                                       sq * 1024 + (u + 1) * 512],
                                start=True, stop=True)
                        at = attnbuf.tile([128, 1024], f16, tag="at", bufs=3,
                                          name=f"at{h}_{sq}_{jt}")
                        nc.scalar.activation(at[:], pp[:], EXP,
                                             bias=0.0, scale=SCALE)
                        ats[jt] = at
                        if jt >= 1:
                            av(jt - 1)
                        if inline_v and jt < JT - 1:
                            v_unit(jt + 1)
                        if interleave:
                            want = (u_idx * JT + jt + 1) * interleave[0] \
                                // (pace * JT)
                            while interleave[0] - len(pending) < want and pending:
                                pending.pop(0)()
                    av(JT - 1)
                    # evacuate po with two quick copies so the next unit's AV
                    # chain gets the PSUM bank back ASAP; the reciprocal and
                    # the normalizing multiply run off the critical path.
                    # (reciprocal_approx_fast mis-reads partition-offset PSUM
                    # APs, and SBUF*SBUF tensor ops need equal input base
                    # partitions, so both operands stage at partitions 0..63.)
                    dstc = ctx[hp][p * 64:(p + 1) * 64,
                                   sq * 1024:(sq + 1) * 1024]
                    ss = attnbuf.tile([64, 1024], f32, tag="ss", bufs=1,
                                      name=f"ss{h}_{sq}")
                    rb = attnbuf.tile([64, 1024], f32, tag="rb", bufs=1,
                                      name=f"rb{h}_{sq}")
                    if last:
                        nc.vector.tensor_copy(ss[:], po[64:128, :])
                        nc.vector.reciprocal_approx_fast(rb[:], ss[:])
                        nc.vector.tensor_mul(dstc, po[0:64, :], rb[:])
                        return
                    cu = attnbuf.tile([64, 1024], f16, tag="cu", bufs=1,
                                      name=f"cu{h}_{sq}")
                    nc.vector.tensor_copy(cu[:], po[0:64, :])
                    nc.vector.tensor_copy(ss[:], po[64:128, :])
                    nc.vector.reciprocal_approx_fast(rb[:], ss[:])
                    nc.vector.tensor_mul(dstc, cu[:], rb[:])

                def kq_units_for(m):
                    return ([lambda s=s, m=m: kq_unit(wt_k[m], kT[m], s)
                             for s in range(4)]
                            + [lambda s=s, m=m: kq_unit(wt_q[m], qT[m], s)
                               for s in range(4)])

                # P0: just enough for attention unit (h=0, sq=0)
                for s in range(4):
                    kq_unit(wt_k[0], kT[0], s)
                kq_unit(wt_q[0], qT[0], 0)
                kq_unit(wt_q[0], qT[0], 1)
                v_unit(0)

                # groups 0..2; group g runs heads (2g, 2g+1), both query
                # halves; P(g+1) threads through A(g)'s slots.  unit order
                # (2g,0),(2g+1,0),(2g,1),(2g+1,1) so qT[g] s2/s3 (drained
                # during the first two units) are ready for the sq=1 pair.
                for g in range(3):
                    if g == 0:
                        pending.extend(
                            [lambda: kq_unit(wt_q[0], qT[0], 2),
                             lambda: kq_unit(wt_q[0], qT[0], 3)]
                            + kq_units_for(1))
                    else:
                        pending.extend(kq_units_for(g + 1))
                    units = [(2 * g, 0), (2 * g + 1, 0),
                             (2 * g, 1), (2 * g + 1, 1)]
                    for i, (h, sq) in enumerate(units):
                        if g == 0 and i == 0:
                            attn_unit(h, sq, None, 0, inline_v=True)
                        elif g == 0:
                            attn_unit(h, sq, [len(pending)] if pending else None,
                                      i - 1, pace=3)
                        else:
                            attn_unit(h, sq, [len(pending)] if pending else None,
                                      i, pace=4)
                    while pending:
                        pending.pop(0)()

                stage_cm.__exit__(None, None, None)
                # stage (x^T, wv) is gone; w_out + the 3-ct half of the output
                # projection overlap the last attention group, the 1-ct tail
                # trails it (overlapped with the output DMA).
                with tc.tile_pool(name="outw", bufs=1) as outw:
                    wo = [outw.tile([128, DIM], f16, tag="wo", bufs=MT,
                                    name=f"wo{ft}") for ft in range(MT)]
                    yps = [outw.tile([128, DIM], f16, tag="yps", bufs=JT,
                                     name=f"yps{qt}") for qt in range(JT)]
                    for ft in range(MT):
                        nc.sync.dma_start(
                            wo[ft][:], WO[ft * 128:(ft + 1) * 128, :])

                    def yp_unit(qt, s, f0, f1):
                        """y rows qt, cols s*512 += sum(ft in f0..f1) ctx.T @ wo."""
                        ps = psInt.tile([128, 512], f32, tag="pint", bufs=2,
                                        name=f"py{qt}_{s}_{f0}")
                        for ft in range(f0, f1):
                            nc.tensor.matmul(ps[:],
                                             ctx[ft][:, qt * 128:(qt + 1) * 128],
                                             wo[ft][:, s * 512:(s + 1) * 512],
                                             start=(ft == f0), stop=(ft == f1 - 1))
                        sl = slice(s * 512, (s + 1) * 512)
                        if f0 == 0:
                            nc.vector.tensor_add(yps[qt][:, sl], ps[:], bias[:, sl])
                        else:
                            ysb = outw.tile([128, 512], f16, tag="ysb", bufs=3,
                                            name=f"ysb{qt}_{s}")
                            nc.vector.tensor_add(ysb[:], ps[:], yps[qt][:, sl])
                            nc.sync.dma_start(
                                Y[qt * 128:(qt + 1) * 128, sl], ysb[:])

                    pending.extend(lambda qt=qt, s=s: yp_unit(qt, s, 0, 3)
                                   for qt in range(JT) for s in (0, 1))
                    ileave = [len(pending)]
                    units = [(6, 0), (7, 0), (6, 1), (7, 1)]
                    for i, (h, sq) in enumerate(units):
                        attn_unit(h, sq, ileave, i, pace=3,
                                  last=(i == len(units) - 1))
                    while pending:
                        pending.pop(0)()
                    # tail: last contraction tile (heads 6,7) + bias'd partial
                    for qt in range(JT):
                        for s in (0, 1):
                            yp_unit(qt, s, 3, 4)

    nc.compile()
    return nc


def _get_compiled():
    global _compiled
    if _compiled is None:
        _compiled = _build()
    return _compiled


def _build_in_maps(x, w_qkv, w_out, b_out):
    x = np.asarray(x, dtype=np.float32)
    w_qkv = np.asarray(w_qkv, dtype=np.float16)
    w_out = np.asarray(w_out, dtype=np.float16)
    b_out = np.asarray(b_out, dtype=np.float32)
    zeros = np.zeros_like(b_out)

    xbs = [np.ascontiguousarray(x[b].T.astype(np.float16)) for b in range(B)]
    in_maps = []
    for c in range(NC):
        b, hg = divmod(c, 2)
        cols = slice(hg * NH * DH, (hg + 1) * NH * DH)
        in_maps.append({
            "x": xbs[b],
            "w_q": np.ascontiguousarray(w_qkv[:, 0 * DIM:1 * DIM][:, cols]),
            "w_k": np.ascontiguousarray(w_qkv[:, 1 * DIM:2 * DIM][:, cols]),
            "w_v": np.ascontiguousarray(w_qkv[:, 2 * DIM:3 * DIM][:, cols]),
            "w_out": np.ascontiguousarray(w_out[cols, :]),
            "b_out": b_out if hg == 0 else zeros,
        })
    return in_maps


def kernel(x, w_qkv, w_out, b_out):
    from concourse.bass_utils import run_bass_kernel_spmd

    nc = _get_compiled()
    in_maps = _build_in_maps(x, w_qkv, w_out, b_out)
    res = run_bass_kernel_spmd(nc, in_maps, core_ids=list(range(NC)))

    out = np.empty((B, N, DIM), dtype=np.float32)
    for b in range(B):
        out[b] = (res.results[2 * b]["y"].astype(np.float32)
                  + res.results[2 * b + 1]["y"].astype(np.float32))
    return out


# revision 11
# speedup vs baseline: 1.3323x; 1.0280x over previous
"""Self-contained Bass/Trainium2 kernel for nn_Attention (B=4, N=2048, D=1024, H=16, dh=64).

Sharding: 8 cores = (batch b in 0..3) x (head-group hg in 0..1), i.e. tensor
parallelism over heads inside each batch pair (per the to_qkv column / to_out
row sharding hint). Each core projects q/k/v for its 8 heads over the FULL
sequence (so no K/V duplication and no cross-core communication), runs
attention for those heads over all 2048 queries, and computes the row-sharded
half of the output projection. The two partial [2048, 1024] outputs of a pair
are summed on the host during unshard (bias is added on the hg=0 core; the
hg=1 core receives a zero bias so one SPMD program serves all cores).

This head split halves the K and V projection matmul work per core vs the
seq-split variant (which must build full-sequence K/V on both pair cores):
PE streaming work drops from ~918K to ~786K cycles/core.

Layout: all matmul operands fp16 (PSUM f32). V is projected directly in
keys-major layout (stationary = x^T blocks, moving = Wv) so no PE transposes
are needed. Each per-head V block carries 64 ones columns, so the AV matmul
emits the softmax row-sums replicated across PSUM partitions 64..127 for
free; the normalization is then one reciprocal_approx_fast + one multiply on
DVE. Projection work is split into PSUM-tile-sized units and interleaved into
the attention loop (V production streams inside the first attention unit at
one key-tile per step, just ahead of its consumption by the AV chain). The
output projection is split 3+1 over its 4 contraction tiles: the 3-tile half
(heads 0-5, ready after group 2) threads through the last attention group;
only the 1-tile tail trails the attention, overlapped with the output DMA.
"""

import sys
import numpy as np

sys.path.insert(0, "/opt/trn_rl_repo")

B, N, DIM = 4, 2048, 1024
HEADS, DH = 16, 64
NH = 8                # heads per core
SCALE = DH ** -0.5    # 0.125
NC = 8
HALF = N // 2

_compiled = None


def _build():
    import concourse.tile as tile
    from concourse import bacc, mybir

    f32 = mybir.dt.float32
    f16 = mybir.dt.float16
    EXP = mybir.ActivationFunctionType.Exp

    nc = bacc.Bacc("TRN2", target_bir_lowering=False, debug=False, num_devices=NC)

    X = nc.dram_tensor("x", (DIM, N), f16, kind="ExternalInput").ap()
    WQ = nc.dram_tensor("w_q", (DIM, NH * DH), f16, kind="ExternalInput").ap()
    WK = nc.dram_tensor("w_k", (DIM, NH * DH), f16, kind="ExternalInput").ap()
    WV = nc.dram_tensor("w_v", (DIM, NH * DH), f16, kind="ExternalInput").ap()
    WO = nc.dram_tensor("w_out", (NH * DH, DIM), f16, kind="ExternalInput").ap()
    BOUT = nc.dram_tensor("b_out", (DIM,), f32, kind="ExternalInput").ap()
    Y = nc.dram_tensor("y", (N, DIM), f16, kind="ExternalOutput").ap()

    CT = DIM // 128       # 8 contraction tiles over input channels
    MT = NH * DH // 128   # 4 dim tiles (head-pairs) for kT/qT/ctx
    JT = N // 128         # 16 key tiles
    VW = 128              # per-head v block: 64 dims + 64 ones columns

    with tile.TileContext(nc) as tc:
        with tc.tile_pool(name="persist", bufs=1) as persist, \
             tc.tile_pool(name="attnbuf", bufs=1) as attnbuf, \
             tc.tile_pool(name="wpool", bufs=1) as wpool:

            kT = [persist.tile([128, N], f16, tag="kT", bufs=MT, name=f"kT{m}")
                  for m in range(MT)]
            qT = [persist.tile([128, N], f16, tag="qT", bufs=MT,
                               name=f"qT{m}") for m in range(MT)]
            v_ext = [persist.tile([128, NH * VW], f16, tag="vext", bufs=JT,
                                  name=f"vext{t}") for t in range(JT)]
            ctx = [persist.tile([128, N], f16, tag="ctx", bufs=MT,
                                name=f"ctx{m}") for m in range(MT)]

            # bias broadcast to all partitions once (zeros on hg=1 cores)
            bias_src = persist.tile([1, DIM], f32, tag="bias_src")
            nc.sync.dma_start(bias_src[:], BOUT.rearrange("(o d) -> o d", o=1))
            bias = persist.tile([128, DIM], f32, tag="bias")
            nc.gpsimd.partition_broadcast(bias[:], bias_src[0:1, :])

            # prefire the exp table load off the critical path
            dummy = attnbuf.tile([1, 8], f16, tag="dummy")
            nc.scalar.activation(dummy[:], bias_src[0:1, 0:8], EXP,
                                 bias=0.0, scale=1.0)

            # ones columns of v_ext (disjoint from the V-projection writes)
            for t in range(JT):
                ones_col = v_ext[t].rearrange(
                    "p (hh c) -> p hh c", c=VW)[:, :, DH:VW]
                nc.gpsimd.memset(ones_col, 1.0)

            with tc.tile_pool(name="psB", bufs=1, space="PSUM") as psB, \
                 tc.tile_pool(name="psInt", bufs=1, space="PSUM") as psInt:
                stage_cm = tc.tile_pool(name="stage", bufs=1)
                stage = stage_cm.__enter__()

                def w_col(src, m):
                    """[128, 8, 128] view of src[:, m*128 : (m+1)*128]."""
                    return src[:, m * 128:(m + 1) * 128].rearrange(
                        "(t p) d -> p t d", p=128)

                # ---- weight + x staging, ordered for fastest first matmul ----
                wt_k = [wpool.tile([128, CT, 128], f16, tag="wkq",
                                   bufs=2 * MT, name=f"wk{m}") for m in range(MT)]
                wt_q = [wpool.tile([128, CT, 128], f16, tag="wkq",
                                   bufs=2 * MT, name=f"wq{m}") for m in range(MT)]
                xbT = [stage.tile([128, N], f16, tag="xbT", bufs=CT,
                                  name=f"xbT{ct}") for ct in range(CT)]
                wv = [stage.tile([128, 512], f16, tag="wv", bufs=CT,
                                 name=f"wv{ct}") for ct in range(CT)]

                # first K weight tile: low ct blocks first so the opening
                # matmul chain starts sooner.  x slices split across both
                # HWDGE queues (SP + Activation) — the scalar engine is idle
                # until the first exp, so its queue doubles load bandwidth
                # through the startup phase.
                wc = w_col(WK, 0)
                nc.sync.dma_start(wt_k[0][:, 0:2, :], wc[:, 0:2, :])
                nc.sync.dma_start(wt_k[0][:, 2:CT, :], wc[:, 2:CT, :])

                def dma_x_slice(s, split=False):
                    for ct in range(CT):
                        eng = nc.scalar if split and ct % 2 else nc.sync
                        eng.dma_start(
                            xbT[ct][:, s * 512:(s + 1) * 512],
                            X[ct * 128:(ct + 1) * 128, s * 512:(s + 1) * 512])

                dma_x_slice(0, split=True)
                nc.sync.dma_start(wt_q[0][:], w_col(WQ, 0))
                dma_x_slice(1, split=True)
                for ct in range(CT):
                    nc.sync.dma_start(wv[ct][:],
                                      WV[ct * 128:(ct + 1) * 128, :])
                dma_x_slice(2)
                nc.sync.dma_start(wt_k[1][:], w_col(WK, 1))
                nc.sync.dma_start(wt_q[1][:], w_col(WQ, 1))
                dma_x_slice(3)
                for m in (2, 3):
                    nc.sync.dma_start(wt_k[m][:], w_col(WK, m))
                    nc.sync.dma_start(wt_q[m][:], w_col(WQ, m))

                # ---- projection units: one PSUM-tile lifecycle each ----
                def kq_unit(wt, dst, s):
                    """dst[:, s*512 : +512] = W[:, m-block].T @ x^T."""
                    ps = psInt.tile([128, 512], f32, tag="pint", bufs=2,
                                    name=f"pi_{dst.name}_{s}")
                    for ct in range(CT):
                        nc.tensor.matmul(ps[:],
                                         wt[:, ct, :],
                                         xbT[ct][:, s * 512:(s + 1) * 512],
                                         start=(ct == 0), stop=(ct == CT - 1))
                    nc.vector.tensor_copy(
                        dst[:, s * 512:(s + 1) * 512], ps[:])

                def v_unit(t):
                    """v_ext[t] (all 8 heads) from x^T key block t."""
                    ps = psInt.tile([128, 512], f32, tag="pint", bufs=2,
                                    name=f"pv{t}")
                    for ct in range(CT):
                        nc.tensor.matmul(
                            ps[:],
                            xbT[ct][:, t * 128:(t + 1) * 128],
                            wv[ct][:],
                            start=(ct == 0), stop=(ct == CT - 1))
                    dst = v_ext[t].rearrange("p (hh c) -> p hh c", c=VW)[
                        :, :, 0:DH]
                    nc.vector.tensor_copy(dst, ps.rearrange(
                        "p (hh c) -> p hh c", c=DH))

                pending = []

                # ---- attention for one (head, query-half) unit ----
                def attn_unit(h, sq, interleave, u_idx, pace=4,
                              inline_v=False, last=False):
                    hp, p = divmod(h, 2)
                    po = psB.tile([128, 1024], f32, tag="po", bufs=1,
                                  name=f"po{h}_{sq}")
                    ats = {}

                    def av(j):
                        at = ats.pop(j)
                        for u in (0, 1):
                            nc.tensor.matmul(
                                po[:, u * 512:(u + 1) * 512],
                                v_ext[j][:, h * VW:(h + 1) * VW],
                                at[:, u * 512:(u + 1) * 512],
                                start=(j == 0), stop=(j == JT - 1))

                    # AV runs in lagged pairs (av(jt-3), av(jt-2) at odd jt):
                    # the second av of a pair continues the po accumulation
                    # chain back-to-back, so its LDWEIGHTS pipelines like a
                    # mid-chain load instead of paying the ~95ns group-entry
                    # stall.  Lag 3/2 keeps the exp producer well ahead.
                    for jt in range(JT):
                        pp = psB.tile([128, 1024], f32, tag="pp", bufs=2,
                                      name=f"pp{h}_{sq}_{jt}")
                        for u in (0, 1):
                            nc.tensor.matmul(
                                pp[:, u * 512:(u + 1) * 512],
                                kT[hp][p * 64:(p + 1) * 64,
                                       jt * 128:(jt + 1) * 128],
                                qT[hp][p * 64:(p + 1) * 64,
                                       sq * 1024 + u * 512:
                                       sq * 1024 + (u + 1) * 512],
                                start=True, stop=True)
                        at = attnbuf.tile([128, 1024], f16, tag="at", bufs=4,
                                          name=f"at{h}_{sq}_{jt}")
                        nc.scalar.activation(at[:], pp[:], EXP,
                                             bias=0.0, scale=SCALE)
                        ats[jt] = at
                        if jt % 2 == 1 and jt >= 3:
                            av(jt - 3)
                            av(jt - 2)
                        if inline_v and jt < JT - 1:
                            v_unit(jt + 1)
                        if interleave:
                            want = (u_idx * JT + jt + 1) * interleave[0] \
                                // (pace * JT)
                            while interleave[0] - len(pending) < want and pending:
                                pending.pop(0)()
                    av(JT - 2)
                    av(JT - 1)
                    # evacuate po with two quick copies so the next unit's AV
                    # chain gets the PSUM bank back ASAP; the reciprocal and
                    # the normalizing multiply run off the critical path.
                    # (reciprocal_approx_fast mis-reads partition-offset PSUM
                    # APs, and SBUF*SBUF tensor ops need equal input base
                    # partitions, so both operands stage at partitions 0..63.)
                    dstc = ctx[hp][p * 64:(p + 1) * 64,
                                   sq * 1024:(sq + 1) * 1024]
                    ss = attnbuf.tile([64, 1024], f32, tag="ss", bufs=1,
                                      name=f"ss{h}_{sq}")
                    rb = attnbuf.tile([64, 1024], f32, tag="rb", bufs=1,
                                      name=f"rb{h}_{sq}")
                    if last:
                        nc.vector.tensor_copy(ss[:], po[64:128, :])
                        nc.vector.reciprocal_approx_fast(rb[:], ss[:])
                        nc.vector.tensor_mul(dstc, po[0:64, :], rb[:])
                        return
                    cu = attnbuf.tile([64, 1024], f16, tag="cu", bufs=1,
                                      name=f"cu{h}_{sq}")
                    nc.vector.tensor_copy(cu[:], po[0:64, :])
                    nc.vector.tensor_copy(ss[:], po[64:128, :])
                    nc.vector.reciprocal_approx_fast(rb[:], ss[:])
                    nc.vector.tensor_mul(dstc, cu[:], rb[:])

                def kq_units_for(m):
                    return ([lambda s=s, m=m: kq_unit(wt_k[m], kT[m], s)
                             for s in range(4)]
                            + [lambda s=s, m=m: kq_unit(wt_q[m], qT[m], s)
                               for s in range(4)])

                # P0: just enough for attention unit (h=0, sq=0)
                for s in range(4):
                    kq_unit(wt_k[0], kT[0], s)
                kq_unit(wt_q[0], qT[0], 0)
                kq_unit(wt_q[0], qT[0], 1)
                v_unit(0)

                # groups 0..2; group g runs heads (2g, 2g+1), both query
                # halves; P(g+1) threads through A(g)'s slots.  unit order
                # (2g,0),(2g+1,0),(2g,1),(2g+1,1) so qT[g] s2/s3 (drained
                # during the first two units) are ready for the sq=1 pair.
                for g in range(3):
                    if g == 0:
                        pending.extend(
                            [lambda: kq_unit(wt_q[0], qT[0], 2),
                             lambda: kq_unit(wt_q[0], qT[0], 3)]
                            + kq_units_for(1))
                    else:
                        pending.extend(kq_units_for(g + 1))
                    units = [(2 * g, 0), (2 * g + 1, 0),
                             (2 * g, 1), (2 * g + 1, 1)]
                    for i, (h, sq) in enumerate(units):
                        if g == 0 and i == 0:
                            attn_unit(h, sq, None, 0, inline_v=True)
                        elif g == 0:
                            attn_unit(h, sq, [len(pending)] if pending else None,
                                      i - 1, pace=3)
                        else:
                            attn_unit(h, sq, [len(pending)] if pending else None,
                                      i, pace=4)
                    while pending:
                        pending.pop(0)()

                stage_cm.__exit__(None, None, None)
                # stage (x^T, wv) is gone.  Output projection schedule:
                #  - rows 0..1023 (sq=0 ctx, complete after unit (7,0)):
                #    full 4-ct units + output DMA thread through attention
                #    units (6,1) and (7,1), so half the output DMA streams
                #    during the last attention stretch.
                #  - rows 1024..2047: the 3-ct part (heads 0..5, ready at
                #    group-3 entry) threads through units (6,0)/(7,0) into
                #    yps; only the 1-ct ctx[3] tail trails the attention,
                #    overlapped with the remaining output DMA.
                with tc.tile_pool(name="outw", bufs=1) as outw:
                    wo = [outw.tile([128, DIM], f16, tag="wo", bufs=MT,
                                    name=f"wo{ft}") for ft in range(MT)]
                    yps = [outw.tile([128, DIM], f16, tag="yps", bufs=JT // 2,
                                     name=f"yps{qt}") for qt in range(8, JT)]
                    for ft in range(MT):
                        nc.sync.dma_start(
                            wo[ft][:], WO[ft * 128:(ft + 1) * 128, :])

                    def yp_unit(qt, s, f0, f1):
                        """y rows qt, cols s*512 += sum(ft in f0..f1) ctx.T @ wo."""
                        ps = psInt.tile([128, 512], f32, tag="pint", bufs=2,
                                        name=f"py{qt}_{s}_{f0}")
                        for ft in range(f0, f1):
                            nc.tensor.matmul(ps[:],
                                             ctx[ft][:, qt * 128:(qt + 1) * 128],
                                             wo[ft][:, s * 512:(s + 1) * 512],
                                             start=(ft == f0), stop=(ft == f1 - 1))
                        sl = slice(s * 512, (s + 1) * 512)
                        if f0 == 0 and f1 < MT:
                            nc.vector.tensor_add(yps[qt - 8][:, sl], ps[:],
                                                 bias[:, sl])
                            return
                        ysb = outw.tile([128, 512], f16, tag="ysb", bufs=3,
                                        name=f"ysb{qt}_{s}")
                        if f0 == 0:
                            nc.vector.tensor_add(ysb[:], ps[:], bias[:, sl])
                        else:
                            nc.vector.tensor_add(ysb[:], ps[:], yps[qt - 8][:, sl])
                        nc.sync.dma_start(
                            Y[qt * 128:(qt + 1) * 128, sl], ysb[:])

                    pending.extend(lambda qt=qt, s=s: yp_unit(qt, s, 0, 3)
                                   for qt in range(8, JT) for s in (0, 1))
                    ileave = [len(pending)]
                    attn_unit(6, 0, ileave, 0, pace=2)
                    attn_unit(7, 0, ileave, 1, pace=2)
                    while pending:
                        pending.pop(0)()
                    pending.extend(lambda qt=qt, s=s: yp_unit(qt, s, 0, MT)
                                   for qt in range(8) for s in (0, 1))
                    ileave = [len(pending)]
                    attn_unit(6, 1, ileave, 0, pace=2)
                    attn_unit(7, 1, ileave, 1, pace=2, last=True)
                    while pending:
                        pending.pop(0)()
                    # tail: ctx[3] (heads 6,7) contraction for rows 1024..2047
                    for qt in range(8, JT):
                        for s in (0, 1):
                            yp_unit(qt, s, 3, 4)

    nc.compile()
    return nc


def _get_compiled():
    global _compiled
    if _compiled is None:
        _compiled = _build()
    return _compiled


def _build_in_maps(x, w_qkv, w_out, b_out):
    x = np.asarray(x, dtype=np.float32)
    w_qkv = np.asarray(w_qkv, dtype=np.float16)
    w_out = np.asarray(w_out, dtype=np.float16)
    b_out = np.asarray(b_out, dtype=np.float32)
    zeros = np.zeros_like(b_out)

    xbs = [np.ascontiguousarray(x[b].T.astype(np.float16)) for b in range(B)]
    in_maps = []
    for c in range(NC):
        b, hg = divmod(c, 2)
        cols = slice(hg * NH * DH, (hg + 1) * NH * DH)
        in_maps.append({
            "x": xbs[b],
            "w_q": np.ascontiguousarray(w_qkv[:, 0 * DIM:1 * DIM][:, cols]),
            "w_k": np.ascontiguousarray(w_qkv[:, 1 * DIM:2 * DIM][:, cols]),
            "w_v": np.ascontiguousarray(w_qkv[:, 2 * DIM:3 * DIM][:, cols]),
            "w_out": np.ascontiguousarray(w_out[cols, :]),
            "b_out": b_out if hg == 0 else zeros,
        })
    return in_maps


def kernel(x, w_qkv, w_out, b_out):
    from concourse.bass_utils import run_bass_kernel_spmd

    nc = _get_compiled()
    in_maps = _build_in_maps(x, w_qkv, w_out, b_out)
    res = run_bass_kernel_spmd(nc, in_maps, core_ids=list(range(NC)))

    out = np.empty((B, N, DIM), dtype=np.float32)
    for b in range(B):
        out[b] = (res.results[2 * b]["y"].astype(np.float32)
                  + res.results[2 * b + 1]["y"].astype(np.float32))
    return out


# revision 20
# speedup vs baseline: 1.3597x; 1.0206x over previous
"""Self-contained Bass/Trainium2 kernel for nn_Attention (B=4, N=2048, D=1024, H=16, dh=64).

Sharding: 8 cores = (batch b in 0..3) x (head-group hg in 0..1), i.e. tensor
parallelism over heads inside each batch pair (per the to_qkv column / to_out
row sharding hint). Each core projects q/k/v for its 8 heads over the FULL
sequence (so no K/V duplication and no cross-core communication), runs
attention for those heads over all 2048 queries, and computes the row-sharded
half of the output projection. The two partial [2048, 1024] outputs of a pair
are summed on the host during unshard (bias is added on the hg=0 core; the
hg=1 core receives a zero bias so one SPMD program serves all cores).

This head split halves the K and V projection matmul work per core vs the
seq-split variant (which must build full-sequence K/V on both pair cores):
PE streaming work drops from ~918K to ~786K cycles/core.

Layout: all matmul operands fp16 (PSUM f32). V is projected directly in
keys-major layout (stationary = x^T blocks, moving = Wv) so no PE transposes
are needed. Each per-head V block carries 64 ones columns, so the AV matmul
emits the softmax row-sums replicated across PSUM partitions 64..127 for
free; the normalization is then one reciprocal_approx_fast + one multiply on
DVE. Projection work is split into PSUM-tile-sized units and interleaved into
the attention loop (V production streams inside the first attention unit at
one key-tile per step, just ahead of its consumption by the AV chain). The
output projection is split 3+1 over its 4 contraction tiles: the 3-tile half
(heads 0-5, ready after group 2) threads through the last attention group;
only the 1-tile tail trails the attention, overlapped with the output DMA.
"""

import sys
import numpy as np

sys.path.insert(0, "/opt/trn_rl_repo")

B, N, DIM = 4, 2048, 1024
HEADS, DH = 16, 64
NH = 8                # heads per core
SCALE = DH ** -0.5    # 0.125
NC = 8
HALF = N // 2

_compiled = None


def _build():
    import concourse.tile as tile
    from concourse import bacc, mybir

    f32 = mybir.dt.float32
    f16 = mybir.dt.float16
    EXP = mybir.ActivationFunctionType.Exp

    nc = bacc.Bacc("TRN2", target_bir_lowering=False, debug=False, num_devices=NC)

    CT = DIM // 128       # 8 contraction tiles over input channels
    MT = NH * DH // 128   # 4 dim tiles (head-pairs) for kT/qT/ctx
    JT = N // 128         # 16 key tiles
    VW = 128              # per-head v block: 64 dims + 64 ones columns

    # w_q/w_k arrive host-prepacked as [p, m, ct, d] so each m-tile is one
    # contiguous-line DMA (the natural [D, 512] layout would need 256B
    # strided elements — 4x the descriptors and ~4x the issue time).
    X = nc.dram_tensor("x", (DIM, N), f16, kind="ExternalInput").ap()
    WQ = nc.dram_tensor("w_q", (128, MT * CT * 128), f16,
                        kind="ExternalInput").ap()
    WK = nc.dram_tensor("w_k", (128, MT * CT * 128), f16,
                        kind="ExternalInput").ap()
    WV = nc.dram_tensor("w_v", (DIM, NH * DH), f16, kind="ExternalInput").ap()
    WO = nc.dram_tensor("w_out", (NH * DH, DIM), f16, kind="ExternalInput").ap()
    BOUT = nc.dram_tensor("b_out", (DIM,), f32, kind="ExternalInput").ap()
    Y = nc.dram_tensor("y", (N, DIM), f16, kind="ExternalOutput").ap()

    with tile.TileContext(nc) as tc:
        with tc.tile_pool(name="persist", bufs=1) as persist, \
             tc.tile_pool(name="attnbuf", bufs=1) as attnbuf, \
             tc.tile_pool(name="wpool", bufs=1) as wpool:

            kT = [persist.tile([128, N], f16, tag="kT", bufs=MT, name=f"kT{m}")
                  for m in range(MT)]
            qT = [persist.tile([128, N], f16, tag="qT", bufs=MT,
                               name=f"qT{m}") for m in range(MT)]
            v_ext = [persist.tile([128, NH * VW], f16, tag="vext", bufs=JT,
                                  name=f"vext{t}") for t in range(JT)]
            ctx = [persist.tile([128, N], f16, tag="ctx", bufs=MT,
                                name=f"ctx{m}") for m in range(MT)]

            # bias broadcast to all partitions once (zeros on hg=1 cores)
            bias_src = persist.tile([1, DIM], f32, tag="bias_src")
            nc.sync.dma_start(bias_src[:], BOUT.rearrange("(o d) -> o d", o=1))
            bias = persist.tile([128, DIM], f32, tag="bias")
            nc.gpsimd.partition_broadcast(bias[:], bias_src[0:1, :])

            # prefire the exp table load off the critical path
            dummy = attnbuf.tile([1, 8], f16, tag="dummy")
            nc.scalar.activation(dummy[:], bias_src[0:1, 0:8], EXP,
                                 bias=0.0, scale=1.0)

            # ones columns of v_ext (disjoint from the V-projection writes)
            for t in range(JT):
                ones_col = v_ext[t].rearrange(
                    "p (hh c) -> p hh c", c=VW)[:, :, DH:VW]
                nc.gpsimd.memset(ones_col, 1.0)

            with tc.tile_pool(name="psB", bufs=1, space="PSUM") as psB, \
                 tc.tile_pool(name="psInt", bufs=1, space="PSUM") as psInt:
                stage_cm = tc.tile_pool(name="stage", bufs=1)
                stage = stage_cm.__enter__()

                def w_col(src, m):
                    """[128, 8, 128] view of prepacked src for m-tile m."""
                    return src[:, m * CT * 128:(m + 1) * CT * 128].rearrange(
                        "p (t d) -> p t d", d=128)

                # ---- weight + x staging, ordered for fastest first matmul.
                # Few large contiguous-line DMAs: instruction issue (~0.7us
                # per DMA_DIRECT2D) dominates the startup, not bandwidth.
                # x tiles alternate between both HWDGE queues (SP +
                # Activation) — the scalar engine is idle until the first exp.
                wt_k = [wpool.tile([128, CT, 128], f16, tag="wkq",
                                   bufs=2 * MT, name=f"wk{m}") for m in range(MT)]
                wt_q = [wpool.tile([128, CT, 128], f16, tag="wkq",
                                   bufs=2 * MT, name=f"wq{m}") for m in range(MT)]
                xbT = [stage.tile([128, N], f16, tag="xbT", bufs=CT,
                                  name=f"xbT{ct}") for ct in range(CT)]
                wv_all = stage.tile([128, CT, 512], f16, tag="wv")

                nc.sync.dma_start(wt_k[0][:], w_col(WK, 0))
                for ct in range(CT):
                    eng = nc.scalar if ct % 2 else nc.sync
                    eng.dma_start(xbT[ct][:], X[ct * 128:(ct + 1) * 128, :])
                nc.sync.dma_start(wt_q[0][:], w_col(WQ, 0))
                nc.scalar.dma_start(
                    wv_all[:], WV.rearrange("(c p) d -> p c d", p=128))
                nc.sync.dma_start(wt_k[1][:], w_col(WK, 1))
                nc.sync.dma_start(wt_q[1][:], w_col(WQ, 1))
                for m in (2, 3):
                    nc.sync.dma_start(wt_k[m][:], w_col(WK, m))
                    nc.sync.dma_start(wt_q[m][:], w_col(WQ, m))

                # ---- projection units: one PSUM-tile lifecycle each ----
                def kq_unit(wt, dst, s):
                    """dst[:, s*512 : +512] = W[:, m-block].T @ x^T."""
                    ps = psInt.tile([128, 512], f32, tag="pint", bufs=2,
                                    name=f"pi_{dst.name}_{s}")
                    for ct in range(CT):
                        nc.tensor.matmul(ps[:],
                                         wt[:, ct, :],
                                         xbT[ct][:, s * 512:(s + 1) * 512],
                                         start=(ct == 0), stop=(ct == CT - 1))
                    nc.vector.tensor_copy(
                        dst[:, s * 512:(s + 1) * 512], ps[:])

                def v_unit(t):
                    """v_ext[t] (all 8 heads) from x^T key block t."""
                    ps = psInt.tile([128, 512], f32, tag="pint", bufs=2,
                                    name=f"pv{t}")
                    for ct in range(CT):
                        nc.tensor.matmul(
                            ps[:],
                            xbT[ct][:, t * 128:(t + 1) * 128],
                            wv_all[:, ct, :],
                            start=(ct == 0), stop=(ct == CT - 1))
                    dst = v_ext[t].rearrange("p (hh c) -> p hh c", c=VW)[
                        :, :, 0:DH]
                    nc.vector.tensor_copy(dst, ps.rearrange(
                        "p (hh c) -> p hh c", c=DH))

                pending = []

                # ---- attention for one (head, query-half) unit ----
                def attn_unit(h, sq, interleave, u_idx, pace=4,
                              inline_v=False, last=False):
                    hp, p = divmod(h, 2)
                    po = psB.tile([128, 1024], f32, tag="po", bufs=1,
                                  name=f"po{h}_{sq}")
                    ats = {}

                    def av(j):
                        at = ats.pop(j)
                        for u in (0, 1):
                            nc.tensor.matmul(
                                po[:, u * 512:(u + 1) * 512],
                                v_ext[j][:, h * VW:(h + 1) * VW],
                                at[:, u * 512:(u + 1) * 512],
                                start=(j == 0), stop=(j == JT - 1))

                    # AV runs in lagged pairs (av(jt-3), av(jt-2) at odd jt):
                    # the second av of a pair continues the po accumulation
                    # chain back-to-back, so its LDWEIGHTS pipelines like a
                    # mid-chain load instead of paying the ~95ns group-entry
                    # stall.  Lag 3/2 keeps the exp producer well ahead.
                    for jt in range(JT):
                        pp = psB.tile([128, 1024], f32, tag="pp", bufs=2,
                                      name=f"pp{h}_{sq}_{jt}")
                        for u in (0, 1):
                            nc.tensor.matmul(
                                pp[:, u * 512:(u + 1) * 512],
                                kT[hp][p * 64:(p + 1) * 64,
                                       jt * 128:(jt + 1) * 128],
                                qT[hp][p * 64:(p + 1) * 64,
                                       sq * 1024 + u * 512:
                                       sq * 1024 + (u + 1) * 512],
                                start=True, stop=True)
                        at = attnbuf.tile([128, 1024], f16, tag="at", bufs=4,
                                          name=f"at{h}_{sq}_{jt}")
                        nc.scalar.activation(at[:], pp[:], EXP,
                                             bias=0.0, scale=SCALE)
                        ats[jt] = at
                        if jt % 2 == 1 and jt >= 3:
                            av(jt - 3)
                            av(jt - 2)
                        if inline_v and jt < JT - 1:
                            v_unit(jt + 1)
                        if interleave:
                            want = (u_idx * JT + jt + 1) * interleave[0] \
                                // (pace * JT)
                            while interleave[0] - len(pending) < want and pending:
                                pending.pop(0)()
                    av(JT - 2)
                    av(JT - 1)
                    # evacuate po with two quick copies so the next unit's AV
                    # chain gets the PSUM bank back ASAP; the reciprocal and
                    # the normalizing multiply run off the critical path.
                    # (reciprocal_approx_fast mis-reads partition-offset PSUM
                    # APs, and SBUF*SBUF tensor ops need equal input base
                    # partitions, so both operands stage at partitions 0..63.)
                    dstc = ctx[hp][p * 64:(p + 1) * 64,
                                   sq * 1024:(sq + 1) * 1024]
                    ss = attnbuf.tile([64, 1024], f32, tag="ss", bufs=1,
                                      name=f"ss{h}_{sq}")
                    rb = attnbuf.tile([64, 1024], f32, tag="rb", bufs=1,
                                      name=f"rb{h}_{sq}")
                    if last:
                        # split by column halves so the output-projection tail
                        # (which consumes ctx columns in ascending order) can
                        # start ~2us earlier.
                        for c in (0, 1):
                            cs = slice(c * 512, (c + 1) * 512)
                            nc.vector.tensor_copy(ss[:, cs], po[64:128, cs])
                            nc.vector.reciprocal_approx_fast(rb[:, cs], ss[:, cs])
                            nc.vector.tensor_mul(dstc[:, cs], po[0:64, cs],
                                                 rb[:, cs])
                        return
                    cu = attnbuf.tile([64, 1024], f16, tag="cu", bufs=1,
                                      name=f"cu{h}_{sq}")
                    nc.vector.tensor_copy(cu[:], po[0:64, :])
                    nc.vector.tensor_copy(ss[:], po[64:128, :])
                    nc.vector.reciprocal_approx_fast(rb[:], ss[:])
                    nc.vector.tensor_mul(dstc, cu[:], rb[:])

                def kq_units_for(m):
                    return ([lambda s=s, m=m: kq_unit(wt_k[m], kT[m], s)
                             for s in range(4)]
                            + [lambda s=s, m=m: kq_unit(wt_q[m], qT[m], s)
                               for s in range(4)])

                # P0: just enough for attention unit (h=0, sq=0)
                for s in range(4):
                    kq_unit(wt_k[0], kT[0], s)
                kq_unit(wt_q[0], qT[0], 0)
                kq_unit(wt_q[0], qT[0], 1)
                v_unit(0)

                # groups 0..2; group g runs heads (2g, 2g+1), both query
                # halves; P(g+1) threads through A(g)'s slots.  unit order
                # (2g,0),(2g+1,0),(2g,1),(2g+1,1) so qT[g] s2/s3 (drained
                # during the first two units) are ready for the sq=1 pair.
                for g in range(3):
                    if g == 0:
                        pending.extend(
                            [lambda: kq_unit(wt_q[0], qT[0], 2),
                             lambda: kq_unit(wt_q[0], qT[0], 3)]
                            + kq_units_for(1))
                    else:
                        pending.extend(kq_units_for(g + 1))
                    units = [(2 * g, 0), (2 * g + 1, 0),
                             (2 * g, 1), (2 * g + 1, 1)]
                    for i, (h, sq) in enumerate(units):
                        if g == 0 and i == 0:
                            attn_unit(h, sq, None, 0, inline_v=True)
                        elif g == 0:
                            attn_unit(h, sq, [len(pending)] if pending else None,
                                      i - 1, pace=3)
                        else:
                            attn_unit(h, sq, [len(pending)] if pending else None,
                                      i, pace=4)
                    while pending:
                        pending.pop(0)()

                stage_cm.__exit__(None, None, None)
                # stage (x^T, wv) is gone.  Output projection schedule:
                #  - rows 0..1023 (sq=0 ctx, complete after unit (7,0)):
                #    full 4-ct units + output DMA thread through attention
                #    units (6,1) and (7,1), so half the output DMA streams
                #    during the last attention stretch.
                #  - rows 1024..2047: the 3-ct part (heads 0..5, ready at
                #    group-3 entry) threads through units (6,0)/(7,0) into
                #    yps; only the 1-ct ctx[3] tail trails the attention,
                #    overlapped with the remaining output DMA.
                with tc.tile_pool(name="outw", bufs=1) as outw:
                    wo = [outw.tile([128, DIM], f16, tag="wo", bufs=MT,
                                    name=f"wo{ft}") for ft in range(MT)]
                    yps = [outw.tile([128, DIM], f16, tag="yps", bufs=JT // 2,
                                     name=f"yps{qt}") for qt in range(8, JT)]
                    for ft in range(MT):
                        nc.sync.dma_start(
                            wo[ft][:], WO[ft * 128:(ft + 1) * 128, :])

                    def yp_unit(qt, s, f0, f1):
                        """y rows qt, cols s*512 += sum(ft in f0..f1) ctx.T @ wo."""
                        ps = psInt.tile([128, 512], f32, tag="pint", bufs=2,
                                        name=f"py{qt}_{s}_{f0}")
                        for ft in range(f0, f1):
                            nc.tensor.matmul(ps[:],
                                             ctx[ft][:, qt * 128:(qt + 1) * 128],
                                             wo[ft][:, s * 512:(s + 1) * 512],
                                             start=(ft == f0), stop=(ft == f1 - 1))
                        sl = slice(s * 512, (s + 1) * 512)
                        if f0 == 0 and f1 < MT:
                            nc.vector.tensor_add(yps[qt - 8][:, sl], ps[:],
                                                 bias[:, sl])
                            return
                        ysb = outw.tile([128, 512], f16, tag="ysb", bufs=4,
                                        name=f"ysb{qt}_{s}")
                        if f0 == 0:
                            nc.vector.tensor_add(ysb[:], ps[:], bias[:, sl])
                        else:
                            nc.vector.tensor_add(ysb[:], ps[:], yps[qt - 8][:, sl])
                        nc.sync.dma_start(
                            Y[qt * 128:(qt + 1) * 128, sl], ysb[:])

                    # rows 1024..2047: 3-ct partial (heads 0..5) + bias into
                    # yps during units (6,0)/(7,0)
                    pending.extend(lambda qt=qt, s=s: yp_unit(qt, s, 0, 3)
                                   for qt in range(8, JT) for s in (0, 1))
                    ileave = [len(pending)]
                    attn_unit(6, 0, ileave, 0, pace=2)
                    attn_unit(7, 0, ileave, 1, pace=2)
                    while pending:
                        pending.pop(0)()
                    # rows 0..1023: full 4-ct units, DMA'd during (6,1)/(7,1)
                    pending.extend(lambda qt=qt, s=s: yp_unit(qt, s, 0, MT)
                                   for qt in range(8) for s in (0, 1))
                    ileave = [len(pending)]
                    attn_unit(6, 1, ileave, 0, pace=2)
                    attn_unit(7, 1, ileave, 1, pace=2, last=True)
                    while pending:
                        pending.pop(0)()
                    # tail: ctx[3] (heads 6,7) contraction for rows 1024..2047
                    for qt in range(8, JT):
                        for s in (0, 1):
                            yp_unit(qt, s, 3, 4)

    nc.compile()
    return nc


def _get_compiled():
    global _compiled
    if _compiled is None:
        _compiled = _build()
    return _compiled


def _build_in_maps(x, w_qkv, w_out, b_out):
    x = np.asarray(x, dtype=np.float32)
    w_qkv = np.asarray(w_qkv, dtype=np.float16)
    w_out = np.asarray(w_out, dtype=np.float16)
    b_out = np.asarray(b_out, dtype=np.float32)
    zeros = np.zeros_like(b_out)

    def prepack(w):
        # [D, 512] -> [p, m, ct, d] so each m-tile DMA reads contiguous
        # 2KB-per-partition lines on device
        mt, ct = NH * DH // 128, DIM // 128
        return np.ascontiguousarray(
            w.reshape(ct, 128, mt, 128).transpose(1, 2, 0, 3).reshape(
                128, mt * ct * 128))

    xbs = [np.ascontiguousarray(x[b].T.astype(np.float16)) for b in range(B)]
    in_maps = []
    for c in range(NC):
        b, hg = divmod(c, 2)
        cols = slice(hg * NH * DH, (hg + 1) * NH * DH)
        in_maps.append({
            "x": xbs[b],
            "w_q": prepack(w_qkv[:, 0 * DIM:1 * DIM][:, cols]),
            "w_k": prepack(w_qkv[:, 1 * DIM:2 * DIM][:, cols]),
            "w_v": np.ascontiguousarray(w_qkv[:, 2 * DIM:3 * DIM][:, cols]),
            "w_out": np.ascontiguousarray(w_out[cols, :]),
            "b_out": b_out if hg == 0 else zeros,
        })
    return in_maps


def kernel(x, w_qkv, w_out, b_out):
    from concourse.bass_utils import run_bass_kernel_spmd

    nc = _get_compiled()
    in_maps = _build_in_maps(x, w_qkv, w_out, b_out)
    res = run_bass_kernel_spmd(nc, in_maps, core_ids=list(range(NC)))

    out = np.empty((B, N, DIM), dtype=np.float32)
    for b in range(B):
        out[b] = (res.results[2 * b]["y"].astype(np.float32)
                  + res.results[2 * b + 1]["y"].astype(np.float32))
    return out
